# revision 14
# baseline (speedup 1.0000x reference)
"""Trainium2 Bass kernel for nn_ChannelAdder (16x3x512x512 -> 16x20x512x512).

Data-parallel over batch: 16 samples / 8 cores = 2 samples per core.
Per sample, the 512-row image is processed in 5 overlapping 128-row strips
(haloed so all vertical-neighbor ops stay inside one strip; only interior
rows of each strip are written out).

Per strip ([128, 512] tiles, partition = image row, free = image col):
  S1 pointwise: mag, angle (=0.5*atan2 via half-angle + odd-poly atan),
     e_norm, cross product.
  S2 4-neighbor cosine similarity: vertical shifts via PE shift-matrix
     matmuls; dp_down = shift(dp_up); dp_right = column-shifted view of
     dp_left.
  S3 depthwise 3x3 Sobel |grad| over the 10 intermediate channels:
     all vertical taps as PE matmuls (tridiag / first-difference matrices)
     with horizontal taps folded in via column-shifted rhs APs accumulated
     in PSUM; then sqrt(gx^2+gy^2) on ACT/DVE.

All ACT functions used (Square, Sqrt, Abs, Sign, Identity/Copy) live in the
single `sqrt_and_others` table set -> no ACT table switches.
"""
import numpy as np
from contextlib import ExitStack

import concourse.bass as bass
import concourse.bacc as bacc
import concourse.tile as tile
from concourse import mybir
from concourse.bass_utils import run_bass_kernel_spmd

AF = mybir.ActivationFunctionType
OP = mybir.AluOpType
F32 = mybir.dt.float32

N_CORES = 8
B_PER_CORE = 2
H = W = 512
HALF_PI = float(np.pi / 2)
QUARTER_PI = float(np.pi / 4)

# (row_start, valid_row_lo, valid_row_hi) within the strip
STRIPS = [(0, 0, 126), (124, 2, 126), (248, 2, 126), (372, 2, 126), (384, 114, 128)]


def _atan_coeffs(deg=7):
    from numpy.polynomial import chebyshev as C
    zs = (np.cos(np.pi * (np.arange(2000) + 0.5) / 2000) + 1) / 2
    rs = np.sqrt(zs)
    fs = np.arctan(rs) / rs
    ch = C.Chebyshev.fit(zs, fs, deg, domain=[0, 1])
    return [float(v) for v in ch.convert(kind=np.polynomial.Polynomial).coef]


ATAN_Q = _atan_coeffs(7)


def _consts_np():
    up = np.eye(128, k=-1, dtype=np.float32)      # lhsT of UP shift (out[p]=in[p+1])
    dn = np.eye(128, k=+1, dtype=np.float32)      # lhsT of DN shift (out[p]=in[p-1])
    tv = (2 * np.eye(128) + np.eye(128, k=1) + np.eye(128, k=-1)).astype(np.float32)
    dvT = (np.eye(128, k=-1) - np.eye(128, k=1)).astype(np.float32)  # lhsT of Dv
    return np.concatenate([up, dn, tv, -tv, dvT, 2 * dvT], axis=1)   # [128, 768]


def _build_strip(nc, tc, ctx, pools, cb, x_d, o_d, b, s, v0, v1):
    pin, pkeep, pscr, pps2, pps3 = pools
    dt = F32
    T = [128, W]

    def lhsT(k):  # k-th [128,128] const block as lhsT AP
        return cb[:, k * 128:(k + 1) * 128]

    UP_T, DN_T, TV_T, NTV_T, DV_T, DV2_T = 0, 1, 2, 3, 4, 5

    # ---- loads ----
    x0 = pin.tile(T, dt, tag="x0")
    x1 = pin.tile(T, dt, tag="x1")
    xd = pin.tile(T, dt, tag="xd")
    nc.sync.dma_start(out=x0[:], in_=x_d[b, 0, s:s + 128, :])
    nc.sync.dma_start(out=x1[:], in_=x_d[b, 1, s:s + 128, :])
    nc.sync.dma_start(out=xd[:], in_=x_d[b, 2, s:s + 128, :])

    # ---- S1 pointwise ----
    sq0 = pscr.tile(T, dt, tag="t0")
    sq1 = pscr.tile(T, dt, tag="t1")
    nc.scalar.activation(sq0[:], x0[:], AF.Square)
    nc.scalar.activation(sq1[:], x1[:], AF.Square)
    ssum = pscr.tile(T, dt, tag="t2")
    nc.vector.tensor_add(ssum[:], sq0[:], sq1[:])
    mag = pkeep.tile(T, dt, tag="mag")
    nc.scalar.activation(mag[:], ssum[:], AF.Sqrt)
    rmag = pscr.tile(T, dt, tag="t0")
    nc.vector.reciprocal_approx_fast(out=rmag[:], in_=mag[:])
    n0 = pkeep.tile(T, dt, tag="n0")
    n1 = pkeep.tile(T, dt, tag="n1")
    nc.vector.tensor_mul(n0[:], x0[:], rmag[:])
    nc.vector.tensor_mul(n1[:], x1[:], rmag[:])
    cross = pkeep.tile(T, dt, tag="cross")
    nc.gpsimd.tensor_mul(cross[:], x0[:], x1[:])

    # angle = 0.5*atan2(x0, x1): quadrant reduction on |x0|,|x1| (no sqrt
    # dependence, no cancellation), masks via Sign in {-1,0,1}.
    ax = pscr.tile(T, dt, tag="t1")
    ay = pscr.tile(T, dt, tag="t2")
    nc.scalar.activation(ax[:], x1[:], AF.Abs)
    nc.scalar.activation(ay[:], x0[:], AF.Abs)
    sdiff = pscr.tile(T, dt, tag="t3")
    nc.vector.tensor_sub(sdiff[:], ax[:], ay[:])
    num = pscr.tile(T, dt, tag="t4")
    den = pscr.tile(T, dt, tag="t5")
    nc.vector.tensor_tensor(out=num[:], in0=ax[:], in1=ay[:], op=OP.min)
    nc.vector.tensor_tensor(out=den[:], in0=ax[:], in1=ay[:], op=OP.max)
    rden = pscr.tile(T, dt, tag="t6")
    nc.vector.reciprocal_approx_fast(out=rden[:], in_=den[:])
    r = pscr.tile(T, dt, tag="t5")
    nc.vector.tensor_mul(r[:], num[:], rden[:])
    zb1 = pscr.tile(T, dt, tag="t3")
    nc.scalar.activation(zb1[:], sdiff[:], AF.Sign)
    u1 = pscr.tile(T, dt, tag="t4")
    nc.scalar.activation(u1[:], zb1[:], AF.Identity, bias=QUARTER_PI,
                         scale=-QUARTER_PI)
    zb2 = pscr.tile(T, dt, tag="t6")
    nc.scalar.activation(zb2[:], x1[:], AF.Sign)
    u2 = pscr.tile(T, dt, tag="t7")
    nc.scalar.activation(u2[:], zb2[:], AF.Identity, bias=HALF_PI,
                         scale=-HALF_PI)
    sgh = pscr.tile(T, dt, tag="t8")
    nc.scalar.activation(sgh[:], x0[:], AF.Sign)

    # odd poly atan(r) = r * P(r^2) for r in [0,1], P deg-7 Estrin
    q = ATAN_Q
    z = pscr.tile(T, dt, tag="t9")
    nc.scalar.activation(z[:], r[:], AF.Square)
    z2 = pscr.tile(T, dt, tag="t10")
    nc.scalar.activation(z2[:], z[:], AF.Square)
    z4 = pscr.tile(T, dt, tag="t11")
    nc.scalar.activation(z4[:], z2[:], AF.Square)
    b0 = pscr.tile(T, dt, tag="t12")
    b1 = pscr.tile(T, dt, tag="t13")
    b2 = pscr.tile(T, dt, tag="t14")
    b3 = pscr.tile(T, dt, tag="t15")
    nc.scalar.activation(b0[:], z[:], AF.Identity, bias=q[0], scale=q[1])
    nc.scalar.activation(b1[:], z[:], AF.Identity, bias=q[2], scale=q[3])
    nc.scalar.activation(b2[:], z[:], AF.Identity, bias=q[4], scale=q[5])
    nc.scalar.activation(b3[:], z[:], AF.Identity, bias=q[6], scale=q[7])
    m1p = pscr.tile(T, dt, tag="t16")
    nc.vector.tensor_mul(m1p[:], z2[:], b1[:])
    c0 = pscr.tile(T, dt, tag="t12")
    nc.vector.tensor_add(c0[:], b0[:], m1p[:])
    m2p = pscr.tile(T, dt, tag="t13")
    nc.gpsimd.tensor_mul(m2p[:], z2[:], b3[:])
    c1 = pscr.tile(T, dt, tag="t14")
    nc.gpsimd.tensor_add(c1[:], b2[:], m2p[:])
    m3p = pscr.tile(T, dt, tag="t15")
    nc.vector.tensor_mul(m3p[:], z4[:], c1[:])
    P = pscr.tile(T, dt, tag="t16")
    nc.vector.tensor_add(P[:], c0[:], m3p[:])
    t = pscr.tile(T, dt, tag="t9")
    nc.vector.tensor_mul(t[:], r[:], P[:])

    # t1 = t*zb1 + u1 ; t2 = t1*zb2 + u2 ; angle = t2 * 0.5 * sign(x0)
    t1a = pscr.tile(T, dt, tag="t10")
    nc.vector.tensor_mul(t1a[:], t[:], zb1[:])
    t1 = pscr.tile(T, dt, tag="t11")
    nc.gpsimd.tensor_add(t1[:], t1a[:], u1[:])
    t2a = pscr.tile(T, dt, tag="t12")
    nc.gpsimd.tensor_mul(t2a[:], t1[:], zb2[:])
    t2 = pscr.tile(T, dt, tag="t13")
    nc.gpsimd.tensor_add(t2[:], t2a[:], u2[:])
    angle = pkeep.tile(T, dt, tag="angle")
    nc.vector.scalar_tensor_tensor(out=angle[:], in0=t2[:], scalar=0.5,
                                   in1=sgh[:], op0=OP.mult, op1=OP.mult)

    # ---- S2 similarity ----
    p_n0u = pps2.tile(T, dt, tag="ps2")
    p_n1u = pps2.tile(T, dt, tag="ps2")
    nc.tensor.matmul(p_n0u[:], lhsT(UP_T), n0[:], start=True, stop=True)
    nc.tensor.matmul(p_n1u[:], lhsT(UP_T), n1[:], start=True, stop=True)
    m0 = pscr.tile(T, dt, tag="t0")
    m1s = pscr.tile(T, dt, tag="t1")
    nc.vector.tensor_mul(m0[:], n0[:], p_n0u[:])
    nc.vector.tensor_mul(m1s[:], n1[:], p_n1u[:])
    dp_u = pscr.tile(T, dt, tag="t2")
    nc.vector.tensor_add(dp_u[:], m0[:], m1s[:])
    p_dpd = pps2.tile(T, dt, tag="ps2")
    nc.tensor.matmul(p_dpd[:], lhsT(DN_T), dp_u[:], start=True, stop=True)
    mx1 = pscr.tile(T, dt, tag="t3")
    mn1 = pscr.tile(T, dt, tag="t4")
    nc.vector.tensor_tensor(out=mx1[:], in0=dp_u[:], in1=p_dpd[:], op=OP.max)
    nc.vector.tensor_tensor(out=mn1[:], in0=dp_u[:], in1=p_dpd[:], op=OP.min)
    q0 = pscr.tile(T, dt, tag="t5")
    q1 = pscr.tile(T, dt, tag="t6")
    nc.gpsimd.tensor_mul(q0[:, 0:511], n0[:, 0:511], n0[:, 1:512])
    nc.gpsimd.tensor_mul(q1[:, 0:511], n1[:, 0:511], n1[:, 1:512])
    dp_l = pscr.tile(T, dt, tag="t7")
    nc.gpsimd.memset(dp_l[:, 511:512], 0.0)
    nc.gpsimd.tensor_add(dp_l[:, 0:511], q0[:, 0:511], q1[:, 0:511])
    mx2 = pscr.tile(T, dt, tag="t5")
    mn2 = pscr.tile(T, dt, tag="t6")
    nc.vector.tensor_tensor(out=mx2[:], in0=mx1[:], in1=dp_l[:], op=OP.max)
    nc.vector.tensor_tensor(out=mn2[:], in0=mn1[:], in1=dp_l[:], op=OP.min)
    smax = pkeep.tile(T, dt, tag="smax")
    smin = pkeep.tile(T, dt, tag="smin")
    nc.vector.tensor_tensor(out=smax[:, 1:512], in0=mx2[:, 1:512],
                            in1=dp_l[:, 0:511], op=OP.max)
    nc.vector.tensor_tensor(out=smin[:, 1:512], in0=mn2[:, 1:512],
                            in1=dp_l[:, 0:511], op=OP.min)
    nc.vector.tensor_scalar_max(smax[:, 0:1], mx2[:, 0:1], 0.0)
    nc.vector.tensor_scalar_min(smin[:, 0:1], mn2[:, 0:1], 0.0)

    # ---- write out channels 0..9 ----
    ch10 = [x0, x1, mag, angle, n0, n1, cross, xd, smax, smin]
    for i, t in enumerate(ch10):
        nc.sync.dma_start(out=o_d[b, i, s + v0:s + v1, :], in_=t[v0:v1, :])

    # ---- S3 sobel ----
    for i, c in enumerate(ch10):
        pg = pps3.tile([128, 1024], dt, tag="ps3")  # [:,0:512]=gx, [:,512:1024]=gy
        gx = pg[:, 0:512]
        gy = pg[:, 512:1024]
        nc.tensor.matmul(gx[:, 0:511], lhsT(TV_T), c[:, 1:512], start=True, stop=False)
        nc.tensor.matmul(gx[:, 1:512], lhsT(NTV_T), c[:, 0:511], start=False, stop=True)
        nc.tensor.matmul(gy[:, :], lhsT(DV2_T), c[:, :], start=True, stop=False)
        nc.tensor.matmul(gy[:, 1:512], lhsT(DV_T), c[:, 0:511], start=False, stop=False)
        nc.tensor.matmul(gy[:, 0:511], lhsT(DV_T), c[:, 1:512], start=False, stop=True)
        sxy = pscr.tile([128, 1024], dt, tag="sxy")
        nc.scalar.activation(sxy[:], pg[:], AF.Square)
        s2 = pscr.tile(T, dt, tag="s2")
        nc.vector.tensor_add(s2[:], sxy[:, 0:512], sxy[:, 512:1024])
        gm = pscr.tile(T, dt, tag="gm")
        nc.scalar.activation(gm[:], s2[:], AF.Sqrt)
        nc.sync.dma_start(out=o_d[b, 10 + i, s + v0:s + v1, :], in_=gm[v0:v1, :])


def build_nc():
    nc = bacc.Bacc("TRN2", target_bir_lowering=False)
    # register const bias APs used by ACT Identity ops
    for v in (ATAN_Q[0], ATAN_Q[2], ATAN_Q[4], ATAN_Q[6], QUARTER_PI, HALF_PI):
        t = nc.alloc_sbuf_tensor(f"const-f32-q{v}", [128, 1], F32)
        nc.gpsimd.memset(t.ap(), v)
        nc.const_aps.aps[(F32, v)] = t.ap()
    nc.all_engine_barrier()
    x_d = nc.declare_dram_parameter("x", [B_PER_CORE, 3, H, W], F32, isOutput=False)
    o_d = nc.declare_dram_parameter("out", [B_PER_CORE, 20, H, W], F32, isOutput=True)
    c_d = nc.declare_dram_parameter("consts", [128, 768], F32, isOutput=False)
    with tile.TileContext(nc) as tc, ExitStack() as ctx:
        pconst = ctx.enter_context(tc.tile_pool(name="const", bufs=1))
        pin = ctx.enter_context(tc.tile_pool(name="pin", bufs=3))
        pkeep = ctx.enter_context(tc.tile_pool(name="pkeep", bufs=2))
        pscr = ctx.enter_context(tc.tile_pool(name="pscr", bufs=2))
        pps2 = ctx.enter_context(tc.tile_pool(name="pps2", bufs=3, space="PSUM"))
        pps3 = ctx.enter_context(tc.tile_pool(name="pps3", bufs=2, space="PSUM"))
        cb = pconst.tile([128, 768], F32)
        nc.sync.dma_start(out=cb[:], in_=c_d[:, :])
        pools = (pin, pkeep, pscr, pps2, pps3)
        for b in range(B_PER_CORE):
            for (s, v0, v1) in STRIPS:
                _build_strip(nc, tc, ctx, pools, cb, x_d, o_d, b, s, v0, v1)
    nc.compile()
    return nc


_NC_CACHE = None


def _get_nc():
    global _NC_CACHE
    if _NC_CACHE is None:
        _NC_CACHE = build_nc()
    return _NC_CACHE


def run(x, trace=False, **trace_kwargs):
    """x: [16,3,512,512] fp32. Returns (out [16,20,512,512], BassKernelResults)."""
    x = np.ascontiguousarray(np.asarray(x, dtype=np.float32))
    assert x.shape == (N_CORES * B_PER_CORE, 3, H, W)
    nc = _get_nc()
    consts = _consts_np()
    in_maps = [
        {"x": x[c * B_PER_CORE:(c + 1) * B_PER_CORE], "consts": consts}
        for c in range(N_CORES)
    ]
    res = run_bass_kernel_spmd(nc, in_maps, list(range(N_CORES)),
                               trace=trace, **trace_kwargs)
    out = np.concatenate([res.results[c]["out"] for c in range(N_CORES)], axis=0)
    return out, res


def kernel(x):
    out, _ = run(x, trace=False)
    return out
